# revision 1
# baseline (speedup 1.0000x reference)
"""Trainium2 Bass kernel for batched differentiable-MPC (LQR) controller.

Reference semantics: one Riccati backward sweep (time-varying quadratic costs,
shared linear dynamics) + forward rollout, batched over B=512.

Sharding: pure data-parallel, 64 batch elements per core across 8 cores.

Device layout ("GRM"): per core, local batch b = 16*g + s with partition-group
g in [0,4) and free-slot s in [0,16). A per-batch 32x32 z-space matrix M_b and
its augmented linear column q_b live in a [128, 528] f32 tile:
    tile[32*g + i, 33*s + c] = M_b[i, c]   (c < 32)
    tile[32*g + i, 33*s + 32] = q_b[i]

Backward step (V', v' from V, v; all per-batch, z = (x,u), nz=32, nx=24):
    S   = Z^T V          (PE matmul, lhsT = blockdiag(Z) constant)
    J   = S^T (= V Z)    (DVE 32x32 stream transpose; V symmetric)
    Th  = Z^T J          (PE matmul)
    Q   = C_t(+reg) + Th ; qz = q_t + Z^T v
    Gauss-Jordan eliminate the 8 u-rows/cols of [Q | qz] (8 pivots, DVE,
    pivot-row broadcast via per-quadrant stream_shuffle). x-rows of the result
    are [V' | v']; u-rows encode the gains: u = -(row_x . x + row_q)/pivot.
Forward: batch-on-partition [64, *] DVE broadcast-mult + reduce per step.
"""

import os
import sys

import numpy as np

for _p in ("/opt/trn_rl_repo",):
    if _p not in sys.path:
        sys.path.insert(0, _p)

import concourse.bass as bass
import concourse.bacc as bacc
import concourse.mybir as mybir
from concourse import tile
from concourse.bass_utils import run_bass_kernel_spmd

F32 = mybir.dt.float32
AX = mybir.AxisListType
OP = mybir.AluOpType

B, T, NX, NU = 512, 100, 24, 8
NZ = NX + NU  # 32
REG = 1e-6
NCORES = 8
BC = B // NCORES  # 64 batches per core
G, SL = 4, 16  # partition groups x free slots (G*SL = BC)
W = NZ + 1  # 33: block width (32 matrix cols + augmented col)
FW = SL * W  # 528 free width of a GRM tile
XOUT = (T + 1) * NX  # 2424
UOUT = T * NU  # 800
OUT_W = XOUT + UOUT  # 3224

LAST_EXEC_NS = None  # filled when trace info is available

_prog_cache = {}


def _build_program(trace=False):
    """Build the Bass program (one core's SPMD program). Returns nc."""
    nc = bacc.Bacc("TRN2", target_bir_lowering=False, debug=False)

    # DRAM I/O
    caug = nc.dram_tensor("caug", [T, 128, FW], F32, kind="ExternalInput")
    vt0 = nc.dram_tensor("vt0", [128, FW], F32, kind="ExternalInput")
    lz = nc.dram_tensor("lz", [128, 128], F32, kind="ExternalInput")
    masks = nc.dram_tensor("masks", [128, NU * SL], F32, kind="ExternalInput")
    x0p = nc.dram_tensor("x0p", [BC, W], F32, kind="ExternalInput")
    abrep = nc.dram_tensor("abrep", [BC, NX * W], F32, kind="ExternalInput")
    out = nc.dram_tensor("out", [BC, OUT_W], F32, kind="ExternalOutput")
    # internal scratch: per-step gain rows (u-rows of the eliminated tableau)
    kbuf = nc.dram_tensor("kbuf", [T, G, NU, FW], F32)

    with tile.TileContext(nc) as tc:
        with (
            tc.tile_pool(name="const", bufs=1) as cpool,
            tc.tile_pool(name="cstream", bufs=3) as cs_pool,
            tc.tile_pool(name="qa", bufs=2) as qa_pool,
            tc.tile_pool(name="jt", bufs=2) as j_pool,
            tc.tile_pool(name="prow", bufs=2) as pr_pool,
            tc.tile_pool(name="wide", bufs=2) as wide_pool,
            tc.tile_pool(name="small", bufs=3) as sm_pool,
            tc.tile_pool(name="ps_s", bufs=2, space="PSUM") as ps_s,
            tc.tile_pool(name="ps_t", bufs=2, space="PSUM") as ps_t,
            tc.tile_pool(name="ps_v", bufs=2, space="PSUM") as ps_v,
            tc.tile_pool(name="fwd", bufs=1) as f_pool,
            tc.tile_pool(name="kstream", bufs=3) as k_pool,
            tc.tile_pool(name="ftmp", bufs=2) as ft_pool,
        ):
            # ---- constants to SBUF ----
            # bounce matmul operands through a DVE copy: walrus rejects
            # matmuls whose LDWEIGHTS carries >1 sem wait, so make every
            # matmul operand DVE-produced (single wait proc).
            lz_raw = cpool.tile([128, 128], F32, tag="lzraw")
            nc.sync.dma_start(out=lz_raw[:], in_=lz[:])
            lz_t = cpool.tile([128, 128], F32, tag="lz")
            nc.vector.tensor_copy(out=lz_t[:], in_=lz_raw[:])
            mask_t = cpool.tile([128, NU * SL], F32, tag="masks")
            nc.sync.dma_start(out=mask_t[:], in_=masks[:])

            # ---- backward Riccati ----
            vraw = cpool.tile([128, FW], F32, tag="vraw")  # V_T tile (dma)
            nc.sync.dma_start(out=vraw[:], in_=vt0[:])
            vcur = cpool.tile([128, FW], F32, tag="vterm")
            nc.vector.tensor_copy(out=vcur[:], in_=vraw[:])

            def mat_view(t_):  # [128, (SL, 32)] strided matrix-columns view
                return t_[:].rearrange("p (s w) -> p s w", w=W)[:, :, 0:NZ]

            def aug_view(t_):  # [128, SL] augmented column view
                return t_[:].rearrange("p (s w) -> p s w", w=W)[:, :, NZ]

            def col_view(t_, c):  # [128, SL] matrix column c view
                return t_[:].rearrange("p (s w) -> p s w", w=W)[:, :, c]

            for tstep in range(T - 1, -1, -1):
                # stage cost tile (C_t with reg folded | q_t)
                ct = cs_pool.tile([128, FW], F32, tag="ct")
                nc.sync.dma_start(out=ct[:], in_=caug[tstep])

                # S = Z^T V  (PSUM)
                s_ps = ps_s.tile([128, SL * NZ], F32, tag="s")
                nc.tensor.matmul(
                    out=s_ps[:], lhsT=lz_t[:], rhs=mat_view(vcur)
                )
                # J = per-block transpose of S (= V Z), PSUM -> SBUF
                j_sb = j_pool.tile([128, SL * NZ], F32, tag="j")
                nc.vector.transpose(out=j_sb[:], in_=s_ps[:])
                # Th = Z^T J (PSUM)
                th_ps = ps_t.tile([128, SL * NZ], F32, tag="th")
                nc.tensor.matmul(out=th_ps[:], lhsT=lz_t[:], rhs=j_sb[:])
                # vZ = Z^T v (PSUM)
                vz_ps = ps_v.tile([128, SL], F32, tag="vz")
                nc.tensor.matmul(out=vz_ps[:], lhsT=lz_t[:], rhs=aug_view(vcur))

                # Qa = C_t + Th | q_t + vZ
                qa = qa_pool.tile([128, FW], F32, tag="qa")
                nc.vector.tensor_tensor(
                    out=mat_view(qa),
                    in0=mat_view(ct),
                    in1=th_ps[:].rearrange("p (s w) -> p s w", w=NZ),
                    op=OP.add,
                )
                nc.vector.tensor_tensor(
                    out=aug_view(qa), in0=aug_view(ct), in1=vz_ps[:], op=OP.add
                )

                # ---- 8-pivot Gauss-Jordan on the u block ----
                for r in range(NU):
                    pc = NX + r
                    prow = pr_pool.tile([128, FW], F32, tag="prow")
                    nc.vector.stream_shuffle(
                        out=prow[:], in_=qa[:], mask=[pc] * 32
                    )
                    drec = sm_pool.tile([128, SL], F32, tag="drec")
                    nc.vector.reciprocal(out=drec[:], in_=col_view(prow, pc))
                    t1 = sm_pool.tile([128, SL], F32, tag="t1")
                    nc.vector.tensor_tensor(
                        out=t1[:], in0=col_view(qa, pc), in1=drec[:], op=OP.mult
                    )
                    mneg = sm_pool.tile([128, SL], F32, tag="mneg")
                    nc.vector.tensor_tensor(
                        out=mneg[:],
                        in0=mask_t[:, r * SL : (r + 1) * SL],
                        in1=t1[:],
                        op=OP.subtract,
                    )
                    # tmp = mneg (bcast along cols) * prow ; qa += tmp
                    tmp = wide_pool.tile([128, FW], F32, tag="tmp")
                    mneg_b = mneg[:].unsqueeze(2).broadcast_to((128, SL, W))
                    nc.vector.tensor_tensor(
                        out=tmp[:].rearrange("p (s w) -> p s w", w=W),
                        in0=mneg_b,
                        in1=prow[:].rearrange("p (s w) -> p s w", w=W),
                        op=OP.mult,
                    )
                    nc.vector.tensor_tensor(
                        out=qa[:], in0=qa[:], in1=tmp[:], op=OP.add
                    )

                # store gain rows (u-rows) to DRAM for the forward pass
                for g in range(G):
                    nc.sync.dma_start(
                        out=kbuf[tstep, g],
                        in_=qa[32 * g + NX : 32 * g + NZ, :],
                    )
                vcur = qa  # x-rows/cols of qa are [V' | v']

            # ---- forward rollout (batch on partitions) ----
            xt = f_pool.tile([BC, W], F32, tag="xt")  # [x | u(=0) | 1]
            nc.sync.dma_start(out=xt[:], in_=x0p[:])
            ab_t = cpool.tile([BC, NX * W], F32, tag="abrep")
            nc.sync.dma_start(out=ab_t[:], in_=abrep[:])
            xall = f_pool.tile([BC, XOUT], F32, tag="xall")
            uall = f_pool.tile([BC, UOUT], F32, tag="uall")

            for tstep in range(T):
                kt = k_pool.tile([BC, NU * W], F32, tag="kt")
                for g in range(G):
                    nc.sync.dma_start(
                        out=kt[g * SL : (g + 1) * SL, :],
                        in_=kbuf[tstep, g][:].rearrange(
                            "r (s w) -> s r w", w=W
                        ),
                    )
                # record x_t
                nc.scalar.copy(
                    out=xall[:, tstep * NX : (tstep + 1) * NX], in_=xt[:, 0:NX]
                )
                # s = sum_c K[r, c] * xt[c]  (u-slots of xt are zero)
                t0 = ft_pool.tile([BC, NU * W], F32, tag="t0")
                nc.vector.tensor_tensor(
                    out=t0[:].rearrange("p (r w) -> p r w", w=W),
                    in0=kt[:].rearrange("p (r w) -> p r w", w=W),
                    in1=xt[:].unsqueeze(1).broadcast_to((BC, NU, W)),
                    op=OP.mult,
                )
                ssum = ft_pool.tile([BC, NU], F32, tag="ssum")
                nc.vector.tensor_reduce(
                    out=ssum[:],
                    in_=t0[:].rearrange("p (r w) -> p r w", w=W),
                    axis=AX.X,
                    op=OP.add,
                )
                # u = -s / pivot ; pivot at kt[r, NX+r]
                piv = kt[:].rearrange("p (r w) -> p r w", w=W)
                # diagonal view: element (r, NX+r) -> flat offset r*W + NX + r
                pivd = bass.AP(
                    tensor=kt[:].tensor,
                    offset=kt[:].offset + NX,
                    ap=[list(kt[:].ap[0]), [W + 1, NU]],
                )
                prec = ft_pool.tile([BC, NU], F32, tag="prec")
                nc.vector.reciprocal(out=prec[:], in_=pivd)
                ut = ft_pool.tile([BC, NU], F32, tag="ut")
                nc.vector.scalar_tensor_tensor(
                    out=ut[:],
                    in0=ssum[:],
                    scalar=-1.0,
                    in1=prec[:],
                    op0=OP.mult,
                    op1=OP.mult,
                )
                nc.scalar.copy(
                    out=uall[:, tstep * NU : (tstep + 1) * NU], in_=ut[:]
                )
                # z = [x | u | 1]
                zt = ft_pool.tile([BC, W], F32, tag="zt")
                nc.vector.tensor_copy(out=zt[:], in_=xt[:])
                nc.vector.tensor_copy(out=zt[:, NX:NZ], in_=ut[:])
                # x' = [A B 0] z
                t2 = ft_pool.tile([BC, NX * W], F32, tag="t2")
                nc.vector.tensor_tensor(
                    out=t2[:].rearrange("p (i w) -> p i w", w=W),
                    in0=ab_t[:].rearrange("p (i w) -> p i w", w=W),
                    in1=zt[:].unsqueeze(1).broadcast_to((BC, NX, W)),
                    op=OP.mult,
                )
                nc.vector.tensor_reduce(
                    out=xt[:, 0:NX],
                    in_=t2[:].rearrange("p (i w) -> p i w", w=W),
                    axis=AX.X,
                    op=OP.add,
                )
            # final state x_T
            nc.scalar.copy(out=xall[:, T * NX : (T + 1) * NX], in_=xt[:, 0:NX])
            nc.sync.dma_start(out=out[:, 0:XOUT], in_=xall[:])
            nc.sync.dma_start(out=out[:, XOUT:OUT_W], in_=uall[:])

    nc.compile()
    return nc


def _host_pack(inputs):
    """Host-side prep: q precompute + per-core GRM packing. Returns in_maps."""
    x0 = np.asarray(inputs["x0"], np.float32)
    C = np.asarray(inputs["C"], np.float32)
    c = np.asarray(inputs["c"], np.float32)
    C_final = np.asarray(inputs["C_final"], np.float32)
    c_final = np.asarray(inputs["c_final"], np.float32)
    x_ref = np.asarray(inputs["x_ref"], np.float32)
    u_ref = np.asarray(inputs["u_ref"], np.float32)
    A = np.asarray(inputs["A_dyn"], np.float32)
    Bd = np.asarray(inputs["B_dyn"], np.float32)

    zref = np.concatenate([x_ref[:, :T], u_ref], axis=-1)  # [B,T,32]
    q = c - np.matmul(C.reshape(-1, NZ, NZ), zref.reshape(-1, NZ, 1)).reshape(
        B, T, NZ
    )
    VT = C_final[:, :NX, :NX]  # [B,24,24]
    vT = c_final[:, :NX] - np.matmul(
        VT, x_ref[:, -1][..., None]
    ).reshape(B, NX)

    # caug [cores, T, 128, 528]
    # view [cores, T, G, 32, SL, 33]; batch b = core*64 + g*16 + s
    caug = np.zeros((NCORES, T, G, NZ, SL, W), np.float32)
    Cb = C.reshape(NCORES, G, SL, T, NZ, NZ)
    caug[..., 0:NZ] = Cb.transpose(0, 3, 1, 4, 2, 5)
    qb = q.reshape(NCORES, G, SL, T, NZ)
    caug[..., NZ] = qb.transpose(0, 3, 1, 4, 2)
    for k in range(NU):
        caug[:, :, :, NX + k, :, NX + k] += REG
    caug = np.ascontiguousarray(caug.reshape(NCORES, T, 128, FW))

    vt0 = np.zeros((NCORES, G, NZ, SL, W), np.float32)
    VTb = VT.reshape(NCORES, G, SL, NX, NX)
    vt0[:, :, 0:NX, :, 0:NX] = VTb.transpose(0, 1, 3, 2, 4)
    vTb = vT.reshape(NCORES, G, SL, NX)
    vt0[:, :, 0:NX, :, NZ] = vTb.transpose(0, 1, 3, 2)
    vt0 = np.ascontiguousarray(vt0.reshape(NCORES, 128, FW))

    AB = np.concatenate([A, Bd], axis=1)  # [24, 32]
    Zpad = np.zeros((NZ, NZ), np.float32)
    Zpad[0:NX, :] = AB
    lz = np.zeros((128, 128), np.float32)
    for g in range(G):
        lz[32 * g : 32 * g + NZ, 32 * g : 32 * g + NZ] = Zpad

    masks = np.zeros((128, NU * SL), np.float32)
    for r in range(NU):
        for g in range(G):
            masks[32 * g + NX + r, r * SL : (r + 1) * SL] = 1.0

    x0p = np.zeros((NCORES, BC, W), np.float32)
    x0b = x0.reshape(NCORES, G, SL, NX)
    for g in range(G):
        x0p[:, g * SL : (g + 1) * SL, 0:NX] = x0b[:, g]
    x0p[:, :, NZ] = 1.0

    abaug = np.zeros((NX, W), np.float32)
    abaug[:, 0:NZ] = AB
    abrep = np.broadcast_to(abaug.reshape(1, NX * W), (BC, NX * W))
    abrep = np.ascontiguousarray(abrep)

    in_maps = []
    for core in range(NCORES):
        in_maps.append(
            {
                "caug": caug[core],
                "vt0": vt0[core],
                "lz": lz,
                "masks": masks,
                "x0p": x0p[core],
                "abrep": abrep,
            }
        )
    return in_maps


def _unpack(results):
    """results: list of per-core dicts with 'out' [64, 3224] -> [B, 3224]."""
    outs = [results[core]["out"] for core in range(NCORES)]
    full = np.concatenate(outs, axis=0)  # [B, 3224] with b = core*64 + g*16+s
    return full


def kernel(**inputs):
    global LAST_EXEC_NS
    trace = bool(int(os.environ.get("KERNEL_TRACE", "0")))
    key = ("prog", trace)
    if key not in _prog_cache:
        _prog_cache[key] = _build_program(trace=trace)
    nc = _prog_cache[key]
    in_maps = _host_pack(inputs)
    res = run_bass_kernel_spmd(
        nc, in_maps, core_ids=list(range(NCORES)), trace=trace
    )
    LAST_EXEC_NS = res.exec_time_ns
    return _unpack(res.results)


def bench(inputs, iters=10):
    """Device-resident repeated execution timing. Returns best per-call
    wall seconds (execute + dispatch, no host transfers)."""
    import time

    import jax
    from jax.sharding import Mesh, NamedSharding, PartitionSpec
    from jax.experimental.shard_map import shard_map

    from concourse import bass2jax as B2J
    import concourse.mybir as mb

    key = ("prog", False)
    if key not in _prog_cache:
        _prog_cache[key] = _build_program(trace=False)
    nc = _prog_cache[key]
    in_maps = _host_pack(inputs)

    B2J.install_neuronx_cc_hook()
    in_names, out_names, out_avals, zero_outs = [], [], [], []
    for alloc in nc.m.functions[0].allocations:
        if not isinstance(alloc, mybir.MemoryLocationSet):
            continue
        name = alloc.memorylocations[0].name
        if alloc.kind == "ExternalInput":
            if (
                nc.partition_id_tensor is not None
                and name == nc.partition_id_tensor.name
            ):
                continue
            in_names.append(name)
        elif alloc.kind == "ExternalOutput":
            out_names.append(name)
            shape = tuple(alloc.tensor_shape)
            dtype = mybir.dt.np(alloc.dtype)
            out_avals.append(jax.core.ShapedArray(shape, dtype))
            zero_outs.append(np.zeros(shape, dtype))
    n_params = len(in_names)
    all_in_names = list(in_names) + list(out_names)
    partition_name = (
        nc.partition_id_tensor.name if nc.partition_id_tensor else None
    )
    if partition_name is not None:
        all_in_names.append(partition_name)

    def _body(*args):
        operands = list(args)
        if partition_name is not None:
            operands.append(B2J.partition_id_tensor())
        outs = B2J._bass_exec_p.bind(
            *operands,
            out_avals=tuple(out_avals),
            in_names=tuple(all_in_names),
            out_names=tuple(out_names),
            lowering_input_output_aliases=(),
            sim_require_finite=True,
            sim_require_nnan=True,
            nc=nc,
        )
        return tuple(outs)

    devices = jax.devices()[:NCORES]
    mesh = Mesh(np.asarray(devices), ("core",))
    nops = n_params + len(out_names)
    sharded = jax.jit(
        shard_map(
            _body,
            mesh=mesh,
            in_specs=(PartitionSpec("core"),) * nops,
            out_specs=(PartitionSpec("core"),) * len(out_names),
            check_rep=False,
        ),
        keep_unused=True,
    )
    sh = NamedSharding(mesh, PartitionSpec("core"))
    dev_in = [
        jax.device_put(
            np.concatenate(
                [np.asarray(in_maps[c][n]) for c in range(NCORES)], axis=0
            ),
            sh,
        )
        for n in in_names
    ]
    dev_zero = [
        jax.device_put(
            np.zeros((NCORES * z.shape[0], *z.shape[1:]), z.dtype), sh
        )
        for z in zero_outs
    ]
    # warmup (compile)
    outs = sharded(*dev_in, *dev_zero)
    jax.block_until_ready(outs)
    best = float("inf")
    for _ in range(iters):
        t0 = time.perf_counter()
        outs = sharded(*dev_in, *dev_zero)
        jax.block_until_ready(outs)
        best = min(best, time.perf_counter() - t0)
    full = np.asarray(outs[out_names.index("out")])
    return best, full


if __name__ == "__main__":
    # smoke test with random inputs (no reference)
    rng = np.random.default_rng(0)
    demo = {
        "x0": rng.standard_normal((B, NX), dtype=np.float32),
        "C": rng.standard_normal((B, T, NZ, NZ), dtype=np.float32),
        "c": rng.standard_normal((B, T, NZ), dtype=np.float32),
        "C_final": rng.standard_normal((B, NZ, NZ), dtype=np.float32),
        "c_final": rng.standard_normal((B, NZ), dtype=np.float32),
        "x_ref": rng.standard_normal((B, T + 1, NX), dtype=np.float32),
        "u_ref": rng.standard_normal((B, T, NU), dtype=np.float32),
        "A_dyn": rng.standard_normal((NX, NX), dtype=np.float32),
        "B_dyn": rng.standard_normal((NX, NU), dtype=np.float32),
    }
    out = kernel(**demo)
    print("out", out.shape, out.dtype)

